# revision 18
# baseline (speedup 1.0000x reference)
"""Trainium2 Bass kernel for nn_ContrastiveLoss3DTo2D.

Reference computation (B=256, D=1024, margin=0.2):
    scores[i, j] = dot(im[j], s[i, j])                    # [B, B]
    cost_s  = sum_i relu(margin + max_{j!=i} scores[i,j] - scores[i,i])
    cost_im = sum_j relu(margin + max_{i!=j} scores[i,j] - scores[j,j])
    loss = cost_s + cost_im

Sharding: s (and the score matrix) is sharded along i across 8 cores
(32 rows each); im is replicated. Inputs are cast to fp16 on the host
(loss tolerance is 2e-2; fp16 keeps the dot-product error ~1e-4 rel)
which halves HBM traffic — the binding constraint — and doubles DVE
elementwise throughput. Each core streams its 16 MB shard and computes
the 32x256 score block with ONE fused DVE pass per (row, half):
tensor_tensor_reduce does multiply + free-axis accumulate (fp32) in a
single instruction, leaving ACT/PE idle.

Column layout: j = 2p + u (partition p in [0,128), u in {0,1}) so each
DMA descriptor is a contiguous 4 KB run (two adjacent j rows of D).
Per-core reductions produce colmax[256] (diag masked), diag[32], and
rowcost[32], packed via 32x32 stream transposes into a single [4,128]
fp32 tensor written with one 4-descriptor DMA (per-partition-column
outputs would emit hundreds of 4-byte descriptors that crawl for >10us).
The host combines per-core partials exactly as relu/max commute.
"""

import numpy as np

B = 256
D = 1024
M = 8            # cores
BL = B // M      # 32 local rows per core
P = 128          # SBUF partitions
U = 2            # j = 2p + u column interleave
MARGIN = 0.2
NEG = -1.0e30    # diagonal mask value
NEG_INIT = -3.0e38

_NC = None


def _build_nc():
    import concourse.bacc as bacc
    from concourse import mybir
    from concourse.tile import TileContext

    f32 = mybir.dt.float32
    f16 = mybir.dt.float16
    add = mybir.AluOpType.add
    mult = mybir.AluOpType.mult
    amax = mybir.AluOpType.max

    nc = bacc.Bacc(None, target_bir_lowering=False, debug=False)
    im_d = nc.declare_dram_parameter("im", [B, D], f16, isOutput=False)
    s_d = nc.declare_dram_parameter("s", [BL, B, D], f16, isOutput=False)
    mt_d = nc.declare_dram_parameter("mask_t_neg", [P, U * BL], f32, isOutput=False)
    nr_d = nc.declare_dram_parameter("neg_rows", [BL, B], f32, isOutput=False)
    er_d = nc.declare_dram_parameter("eye_rows", [BL, B], f32, isOutput=False)
    o_d = nc.declare_dram_parameter("out", [4, P], f32, isOutput=True)

    with TileContext(nc) as tc:
        with (
            tc.tile_pool(name="const", bufs=1) as cpool,
            tc.tile_pool(name="sload", bufs=4) as spool,
            tc.tile_pool(name="scratch", bufs=2) as prpool,
            tc.tile_pool(name="prods", bufs=6) as mpool,
            tc.tile_pool(name="small", bufs=1) as smpool,
            tc.psum_pool(name="pgarbF", bufs=1) as gpoolF,
            tc.psum_pool(name="pgarbA", bufs=1) as gpoolA,
        ):
            # im packed as [p, u*D + d] with j = 2p + u, matching s tiles.
            # Rides the ACT HWDGE ring with the masks; s opens on Sync.
            im_t = cpool.tile([P, U * D], f16, tag="im")
            nc.scalar.dma_start(
                out=im_t[:].rearrange("p (u d) -> p u d", u=U),
                in_=im_d[:].rearrange("(p u) d -> p u d", p=P),
            )
            mt_t = cpool.tile([P, U * BL], f32, tag="maskT")
            nc.scalar.dma_start(out=mt_t[:], in_=mt_d[:])
            nr_t = cpool.tile([BL, B], f32, tag="negrows")
            nc.scalar.dma_start(out=nr_t[:], in_=nr_d[:])
            er_t = cpool.tile([BL, B], f32, tag="eyerows")
            nc.scalar.dma_start(out=er_t[:], in_=er_d[:])

            # scores^T: partition p, free column u*BL + i  (j = 2p + u)
            scoresT = smpool.tile([P, U * BL], f32, tag="scoresT")

            # Ramped chunks: tiny first so the first fused pass starts
            # ~1.5us in; tiny last so the stream tail is short.
            chunk_rows = [1, 1, 2, 4, 4, 4, 4, 4, 4, 2, 1, 1]
            assert sum(chunk_rows) == BL

            # Per-score engine assignment, measured on HW:
            #   F: DVE fused scalar_tensor_tensor, 1223 ns + 140 acc-read.
            #      Its full-product out is garbage -> PSUM bank (accum
            #      paths run 1 elem/cyc regardless of out placement).
            #   A: DVE fp16 mul -> fp16 SBUF prod (2 elem/cyc, 680 ns;
            #      a f32 or PSUM out would force 1 elem/cyc), then ACT
            #      accum (1148 + 283 read) with its write-back aimed at
            #      a separate PSUM bank to spare SBUF bandwidth.
            # 24 F + 40 A balances DVE and ACT at ~59 us. GpSimd assist
            # was a net loss: its SBUF traffic broke DVE's 2-ports mode.
            # First/last chunks stay fused for short pipeline ends.
            modes = ["F", "F"] + ["A", "A", "F"] * 20 + ["F", "F"]

            row0 = 0
            sidx = 0
            for ci, nr in enumerate(chunk_rows):
                s_t = spool.tile([P, nr * U * D], f16, tag="s")
                ring = nc.sync if ci % 2 == 0 else nc.scalar
                ring.dma_start(
                    out=s_t[:, 0:nr * U * D].rearrange(
                        "p (r f) -> p r f", r=nr
                    ),
                    in_=s_d[row0:row0 + nr].rearrange(
                        "r (p u) d -> p r (u d)", p=P
                    ),
                )
                for r in range(nr):
                    i = row0 + r
                    for u in range(U):
                        off = (r * U + u) * D
                        mode = modes[sidx]
                        sidx += 1
                        acc = scoresT[:, u * BL + i:u * BL + i + 1]
                        if mode == "F":
                            garb = gpoolF.tile([P, D], f32, tag="garbF")
                            nc.vector.scalar_tensor_tensor(
                                out=garb[:],
                                in0=s_t[:, off:off + D],
                                scalar=1.0,
                                in1=im_t[:, u * D:(u + 1) * D],
                                op0=mult,
                                op1=mult,
                                accum_out=acc,
                            )
                        else:
                            prod = mpool.tile([P, D], f16, tag="prod")
                            garbA = gpoolA.tile([P, D], f32, tag="garbA")
                            nc.vector.tensor_mul(
                                prod[:],
                                s_t[:, off:off + D],
                                im_t[:, u * D:(u + 1) * D],
                            )
                            nc.scalar.activation(
                                out=garbA[:],
                                in_=prod[:],
                                func=mybir.ActivationFunctionType.Copy,
                                accum_out=acc,
                            )
                row0 += nr

            # Packed output tile: col 0/1 = colmax (u=0/1), col 2 = diag,
            # col 3 = rowcost. Transposed at the end into [4, 128]. The
            # memset covers the pad lanes the transposes read.
            out_t = smpool.tile([P, 32], f32, tag="out_t")
            nc.gpsimd.memset(out_t[:], 0.0)

            # Column maxima over local rows, diagonal masked to -1e30:
            # fused (scoresT + mask) then max-reduce.
            for u in range(U):
                cscr = prpool.tile([P, BL], f32, tag="cscr")
                nc.vector.tensor_add(
                    cscr[:],
                    scoresT[:, u * BL:(u + 1) * BL],
                    mt_t[:, u * BL:(u + 1) * BL],
                )
                nc.vector.reduce_max(
                    out_t[:, u:u + 1], cscr[:], axis=mybir.AxisListType.X
                )

            # Transpose scores^T -> rows [32, 256] via 32x32 stream blocks.
            # rows[i, u*128 + pp] = score(i, j=2*pp+u).
            rows = smpool.tile([BL, B], f32, tag="rows")
            for u in range(U):
                for k in range(P // 32):
                    nc.vector.transpose(
                        out=rows[0:BL, u * P + k * 32:u * P + (k + 1) * 32],
                        in_=scoresT[k * 32:(k + 1) * 32, u * BL:(u + 1) * BL],
                    )

            # rowmax (diag masked) and diag, both fused single passes.
            rowstat = smpool.tile([BL, 4], f32, tag="rowstat")
            rscr1 = prpool.tile([BL, B], f32, tag="rscr")
            nc.vector.tensor_add(rscr1[:], rows[:], nr_t[:])
            nc.vector.reduce_max(
                rowstat[:, 0:1], rscr1[:], axis=mybir.AxisListType.X
            )
            rscr2 = prpool.tile([BL, B], f32, tag="rscr")
            # diag = sum(rows * eye) fused in one pass
            nc.vector.scalar_tensor_tensor(
                out=rscr2[:],
                in0=rows[:],
                scalar=1.0,
                in1=er_t[:],
                op0=mult,
                op1=mult,
                accum_out=out_t[0:BL, 2:3],
            )
            # rowcost = relu(margin + rowmax - diag)
            nc.vector.tensor_sub(rowstat[:, 1:2], rowstat[:, 0:1], out_t[0:BL, 2:3])
            nc.vector.tensor_scalar(
                out=out_t[0:BL, 3:4], in0=rowstat[:, 1:2],
                scalar1=MARGIN, scalar2=0.0, op0=add, op1=amax,
            )

            # Pack: transpose out_t's first 4 columns into rows of outT,
            # then ONE 4-descriptor DMA (512B per partition line).
            outT = smpool.tile([32, P], f32, tag="outT")
            for k in range(P // 32):
                nc.vector.transpose(
                    out=outT[0:32, k * 32:(k + 1) * 32],
                    in_=out_t[k * 32:(k + 1) * 32, 0:32],
                )
            nc.scalar.dma_start(out=o_d[:], in_=outT[0:4, 0:P])

    nc.compile()
    return nc


def _get_nc():
    global _NC
    if _NC is None:
        _NC = _build_nc()
    return _NC


def _make_in_maps(im, s):
    im16 = im.astype(np.float16)
    s16 = s.astype(np.float16)
    il = np.arange(BL)
    # column q in `rows` layout: q = u*128 + pp  <->  j = 2*pp + u
    jq = 2 * (np.arange(B) % P) + (np.arange(B) // P)
    in_maps = []
    for c in range(M):
        jdiag = c * BL + il                      # global row index of local i
        mt = np.zeros((P, U * BL), np.float32)   # mt[p, u*BL+i]
        pd, ud = jdiag % P, jdiag // P
        # j = 2p+u == jdiag  =>  p = jdiag//2, u = jdiag%2
        mt[jdiag // 2, (jdiag % 2) * BL + il] = NEG
        nr = np.zeros((BL, B), np.float32)
        er = np.zeros((BL, B), np.float32)
        qdiag = (jdiag % 2) * P + jdiag // 2     # q with j(q) == jdiag
        nr[il, qdiag] = NEG
        er[il, qdiag] = 1.0
        in_maps.append({
            "im": im16,
            "s": s16[c * BL:(c + 1) * BL],
            "mask_t_neg": mt,
            "neg_rows": nr,
            "eye_rows": er,
        })
    return in_maps


def _combine(results):
    colmax = np.full(B, -np.inf, np.float32)
    rowcosts = np.empty(B, np.float32)
    diag = np.empty(B, np.float32)
    for c in range(M):
        o = results[c]["out"]                    # [4, 128] fp32
        cm = np.stack([o[0], o[1]], axis=1).ravel()   # j = 2p+u
        colmax = np.maximum(colmax, cm)
        diag[c * BL:(c + 1) * BL] = o[2, :BL]
        rowcosts[c * BL:(c + 1) * BL] = o[3, :BL]
    cost_im = np.maximum(np.float32(MARGIN) + colmax - diag, np.float32(0.0))
    loss = rowcosts.sum(dtype=np.float32) + cost_im.sum(dtype=np.float32)
    return np.array(loss, dtype=np.float32)


def _run(im, s, **spmd_kwargs):
    from concourse.bass_utils import run_bass_kernel_spmd

    im = np.ascontiguousarray(np.asarray(im), dtype=np.float32)
    s = np.ascontiguousarray(np.asarray(s), dtype=np.float32)
    nc = _get_nc()
    res = run_bass_kernel_spmd(nc, _make_in_maps(im, s), list(range(M)),
                               **spmd_kwargs)
    return _combine(res.results), res


def kernel(im, s):
    loss, _ = _run(im, s)
    return loss


# revision 21
# speedup vs baseline: 1.0247x; 1.0247x over previous
"""Trainium2 Bass kernel for nn_ContrastiveLoss3DTo2D.

Reference computation (B=256, D=1024, margin=0.2):
    scores[i, j] = dot(im[j], s[i, j])                    # [B, B]
    cost_s  = sum_i relu(margin + max_{j!=i} scores[i,j] - scores[i,i])
    cost_im = sum_j relu(margin + max_{i!=j} scores[i,j] - scores[j,j])
    loss = cost_s + cost_im

Sharding: s (and the score matrix) is sharded along i across 8 cores
(32 rows each); im is replicated. Inputs are cast to fp16 on the host
(loss tolerance is 2e-2; fp16 keeps the dot-product error ~1e-4 rel)
which halves HBM traffic — the binding constraint — and doubles DVE
elementwise throughput. Each core streams its 16 MB shard and computes
the 32x256 score block with ONE fused DVE pass per (row, half):
tensor_tensor_reduce does multiply + free-axis accumulate (fp32) in a
single instruction, leaving ACT/PE idle.

Column layout: j = 2p + u (partition p in [0,128), u in {0,1}) so each
DMA descriptor is a contiguous 4 KB run (two adjacent j rows of D).
Per-core reductions produce colmax[256] (diag masked), diag[32], and
rowcost[32], packed via 32x32 stream transposes into a single [4,128]
fp32 tensor written with one 4-descriptor DMA (per-partition-column
outputs would emit hundreds of 4-byte descriptors that crawl for >10us).
The host combines per-core partials exactly as relu/max commute.
"""

import numpy as np

B = 256
D = 1024
M = 8            # cores
BL = B // M      # 32 local rows per core
P = 128          # SBUF partitions
U = 2            # j = 2p + u column interleave
MARGIN = 0.2
NEG = -1.0e30    # diagonal mask value
NEG_INIT = -3.0e38

_NC = None


def _build_nc():
    import concourse.bacc as bacc
    from concourse import mybir
    from concourse.tile import TileContext

    f32 = mybir.dt.float32
    f16 = mybir.dt.float16
    add = mybir.AluOpType.add
    mult = mybir.AluOpType.mult
    amax = mybir.AluOpType.max

    nc = bacc.Bacc(None, target_bir_lowering=False, debug=False)
    im_d = nc.declare_dram_parameter("im", [B, D], f16, isOutput=False)
    s_d = nc.declare_dram_parameter("s", [BL, B, D], f16, isOutput=False)
    mt_d = nc.declare_dram_parameter("mask_t_neg", [P, U * BL], f32, isOutput=False)
    nr_d = nc.declare_dram_parameter("neg_rows", [BL, B], f32, isOutput=False)
    er_d = nc.declare_dram_parameter("eye_rows", [BL, B], f32, isOutput=False)
    o_d = nc.declare_dram_parameter("out", [4, P], f32, isOutput=True)

    with TileContext(nc) as tc:
        with (
            tc.tile_pool(name="const", bufs=1) as cpool,
            tc.tile_pool(name="sload", bufs=4) as spool,
            tc.tile_pool(name="scratch", bufs=2) as prpool,
            tc.tile_pool(name="prods", bufs=6) as mpool,
            tc.tile_pool(name="small", bufs=1) as smpool,
            tc.psum_pool(name="pgarbF", bufs=1) as gpoolF,
            tc.psum_pool(name="pgarbA", bufs=1) as gpoolA,
        ):
            # im packed as [p, u*D + d] with j = 2p + u, matching s tiles.
            # Rides the ACT HWDGE ring with the masks; s opens on Sync.
            im_t = cpool.tile([P, U * D], f16, tag="im")
            nc.scalar.dma_start(
                out=im_t[:].rearrange("p (u d) -> p u d", u=U),
                in_=im_d[:].rearrange("(p u) d -> p u d", p=P),
            )
            mt_t = cpool.tile([P, U * BL], f32, tag="maskT")
            nc.scalar.dma_start(out=mt_t[:], in_=mt_d[:])
            nr_t = cpool.tile([BL, B], f32, tag="negrows")
            nc.scalar.dma_start(out=nr_t[:], in_=nr_d[:])
            er_t = cpool.tile([BL, B], f32, tag="eyerows")
            nc.scalar.dma_start(out=er_t[:], in_=er_d[:])

            # scores^T: partition p, free column u*BL + i  (j = 2p + u).
            # One accumulator per engine: a shared tile would serialize
            # DVE and ACT into convoys (every accum write is ordered
            # against the previous engine's write to the same tile).
            # Both start at 0; each column is written by exactly one
            # engine; the epilogue merges with a single add.
            scoresD = smpool.tile([P, U * BL], f32, tag="scoresD")
            scoresA = smpool.tile([P, U * BL], f32, tag="scoresA")
            nc.gpsimd.memset(scoresD[:], 0.0)
            nc.gpsimd.memset(scoresA[:], 0.0)
            scoresT = smpool.tile([P, U * BL], f32, tag="scoresT")

            # Ramped chunks: tiny first so the first fused pass starts
            # ~1.5us in; tiny last so the stream tail is short.
            chunk_rows = [1, 1, 2, 4, 4, 4, 4, 4, 4, 2, 1, 1]
            assert sum(chunk_rows) == BL

            # Per-ROW engine assignment, measured on HW:
            #   F-row: 2x DVE fused scalar_tensor_tensor (1223 + 140 ns
            #      each, 1 elem/cyc ALU-bound; garbage out -> PSUM).
            #   A-row: ONE DVE fp16 mul [P, 2*D] -> fp16 SBUF (2 elem/cyc,
            #      930 ns; covers both halves), then 2x ACT accum
            #      (1148 + 283 ns each, write-back -> PSUM bank).
            # 13 F-rows / 19 A-rows balances DVE and ACT at ~53 us, just
            # above the ~51 us fp16 HBM stream. First/last rows stay F.
            f_rows = {round(k * (BL - 1) / 12) for k in range(13)}
            assert len(f_rows) == 13 and 0 in f_rows and BL - 1 in f_rows

            row0 = 0
            for ci, nr in enumerate(chunk_rows):
                s_t = spool.tile([P, nr * U * D], f16, tag="s")
                ring = nc.sync if ci % 2 == 0 else nc.scalar
                ring.dma_start(
                    out=s_t[:, 0:nr * U * D].rearrange(
                        "p (r f) -> p r f", r=nr
                    ),
                    in_=s_d[row0:row0 + nr].rearrange(
                        "r (p u) d -> p r (u d)", p=P
                    ),
                )
                for r in range(nr):
                    i = row0 + r
                    if i in f_rows:
                        for u in range(U):
                            off = (r * U + u) * D
                            garb = gpoolF.tile([P, D], f32, tag="garbF")
                            nc.vector.scalar_tensor_tensor(
                                out=garb[:],
                                in0=s_t[:, off:off + D],
                                scalar=1.0,
                                in1=im_t[:, u * D:(u + 1) * D],
                                op0=mult,
                                op1=mult,
                                accum_out=scoresD[:, u * BL + i:u * BL + i + 1],
                            )
                    else:
                        off = r * U * D
                        prod = mpool.tile([P, U * D], f16, tag="prod")
                        nc.vector.tensor_mul(
                            prod[:],
                            s_t[:, off:off + U * D],
                            im_t[:],
                        )
                        for u in range(U):
                            garbA = gpoolA.tile([P, D], f32, tag="garbA")
                            nc.scalar.activation(
                                out=garbA[:],
                                in_=prod[:, u * D:(u + 1) * D],
                                func=mybir.ActivationFunctionType.Copy,
                                accum_out=scoresA[:, u * BL + i:u * BL + i + 1],
                            )
                row0 += nr

            # Merge the two engine accumulators (disjoint columns, 0 else)
            nc.vector.tensor_add(scoresT[:], scoresD[:], scoresA[:])

            # Packed output tile: col 0/1 = colmax (u=0/1), col 2 = diag,
            # col 3 = rowcost. Transposed at the end into [4, 128]. The
            # memset covers the pad lanes the transposes read.
            out_t = smpool.tile([P, 32], f32, tag="out_t")
            nc.gpsimd.memset(out_t[:], 0.0)

            # Column maxima over local rows, diagonal masked to -1e30:
            # fused (scoresT + mask) then max-reduce.
            for u in range(U):
                cscr = prpool.tile([P, BL], f32, tag="cscr")
                nc.vector.tensor_add(
                    cscr[:],
                    scoresT[:, u * BL:(u + 1) * BL],
                    mt_t[:, u * BL:(u + 1) * BL],
                )
                nc.vector.reduce_max(
                    out_t[:, u:u + 1], cscr[:], axis=mybir.AxisListType.X
                )

            # Transpose scores^T -> rows [32, 256] via 32x32 stream blocks.
            # rows[i, u*128 + pp] = score(i, j=2*pp+u).
            rows = smpool.tile([BL, B], f32, tag="rows")
            for u in range(U):
                for k in range(P // 32):
                    nc.vector.transpose(
                        out=rows[0:BL, u * P + k * 32:u * P + (k + 1) * 32],
                        in_=scoresT[k * 32:(k + 1) * 32, u * BL:(u + 1) * BL],
                    )

            # rowmax (diag masked) and diag, both fused single passes.
            rowstat = smpool.tile([BL, 4], f32, tag="rowstat")
            rscr1 = prpool.tile([BL, B], f32, tag="rscr")
            nc.vector.tensor_add(rscr1[:], rows[:], nr_t[:])
            nc.vector.reduce_max(
                rowstat[:, 0:1], rscr1[:], axis=mybir.AxisListType.X
            )
            rscr2 = prpool.tile([BL, B], f32, tag="rscr")
            # diag = sum(rows * eye) fused in one pass
            nc.vector.scalar_tensor_tensor(
                out=rscr2[:],
                in0=rows[:],
                scalar=1.0,
                in1=er_t[:],
                op0=mult,
                op1=mult,
                accum_out=out_t[0:BL, 2:3],
            )
            # rowcost = relu(margin + rowmax - diag)
            nc.vector.tensor_sub(rowstat[:, 1:2], rowstat[:, 0:1], out_t[0:BL, 2:3])
            nc.vector.tensor_scalar(
                out=out_t[0:BL, 3:4], in0=rowstat[:, 1:2],
                scalar1=MARGIN, scalar2=0.0, op0=add, op1=amax,
            )

            # Pack: transpose out_t's first 4 columns into rows of outT,
            # then ONE 4-descriptor DMA (512B per partition line).
            outT = smpool.tile([32, P], f32, tag="outT")
            for k in range(P // 32):
                nc.vector.transpose(
                    out=outT[0:32, k * 32:(k + 1) * 32],
                    in_=out_t[k * 32:(k + 1) * 32, 0:32],
                )
            nc.scalar.dma_start(out=o_d[:], in_=outT[0:4, 0:P])

    nc.compile()
    return nc


def _get_nc():
    global _NC
    if _NC is None:
        _NC = _build_nc()
    return _NC


def _make_in_maps(im, s):
    im16 = im.astype(np.float16)
    s16 = s.astype(np.float16)
    il = np.arange(BL)
    # column q in `rows` layout: q = u*128 + pp  <->  j = 2*pp + u
    jq = 2 * (np.arange(B) % P) + (np.arange(B) // P)
    in_maps = []
    for c in range(M):
        jdiag = c * BL + il                      # global row index of local i
        mt = np.zeros((P, U * BL), np.float32)   # mt[p, u*BL+i]
        pd, ud = jdiag % P, jdiag // P
        # j = 2p+u == jdiag  =>  p = jdiag//2, u = jdiag%2
        mt[jdiag // 2, (jdiag % 2) * BL + il] = NEG
        nr = np.zeros((BL, B), np.float32)
        er = np.zeros((BL, B), np.float32)
        qdiag = (jdiag % 2) * P + jdiag // 2     # q with j(q) == jdiag
        nr[il, qdiag] = NEG
        er[il, qdiag] = 1.0
        in_maps.append({
            "im": im16,
            "s": s16[c * BL:(c + 1) * BL],
            "mask_t_neg": mt,
            "neg_rows": nr,
            "eye_rows": er,
        })
    return in_maps


def _combine(results):
    colmax = np.full(B, -np.inf, np.float32)
    rowcosts = np.empty(B, np.float32)
    diag = np.empty(B, np.float32)
    for c in range(M):
        o = results[c]["out"]                    # [4, 128] fp32
        cm = np.stack([o[0], o[1]], axis=1).ravel()   # j = 2p+u
        colmax = np.maximum(colmax, cm)
        diag[c * BL:(c + 1) * BL] = o[2, :BL]
        rowcosts[c * BL:(c + 1) * BL] = o[3, :BL]
    cost_im = np.maximum(np.float32(MARGIN) + colmax - diag, np.float32(0.0))
    loss = rowcosts.sum(dtype=np.float32) + cost_im.sum(dtype=np.float32)
    return np.array(loss, dtype=np.float32)


def _run(im, s, **spmd_kwargs):
    from concourse.bass_utils import run_bass_kernel_spmd

    im = np.ascontiguousarray(np.asarray(im), dtype=np.float32)
    s = np.ascontiguousarray(np.asarray(s), dtype=np.float32)
    nc = _get_nc()
    res = run_bass_kernel_spmd(nc, _make_in_maps(im, s), list(range(M)),
                               **spmd_kwargs)
    return _combine(res.results), res


def kernel(im, s):
    loss, _ = _run(im, s)
    return loss


# revision 24
# speedup vs baseline: 1.1098x; 1.0831x over previous
"""Trainium2 Bass kernel for nn_ContrastiveLoss3DTo2D.

Reference computation (B=256, D=1024, margin=0.2):
    scores[i, j] = dot(im[j], s[i, j])                    # [B, B]
    cost_s  = sum_i relu(margin + max_{j!=i} scores[i,j] - scores[i,i])
    cost_im = sum_j relu(margin + max_{i!=j} scores[i,j] - scores[j,j])
    loss = cost_s + cost_im

Sharding: s (and the score matrix) is sharded along i across 8 cores
(32 rows each); im is replicated. Inputs are cast to fp16 on the host
(loss tolerance is 2e-2; fp16 keeps the dot-product error ~1e-4 rel)
which halves HBM traffic — the binding constraint — and doubles DVE
elementwise throughput. Each core streams its 16 MB shard and computes
the 32x256 score block with ONE fused DVE pass per (row, half):
tensor_tensor_reduce does multiply + free-axis accumulate (fp32) in a
single instruction, leaving ACT/PE idle.

Column layout: j = 2p + u (partition p in [0,128), u in {0,1}) so each
DMA descriptor is a contiguous 4 KB run (two adjacent j rows of D).
Per-core reductions produce colmax[256] (diag masked), diag[32], and
rowcost[32], packed via 32x32 stream transposes into a single [4,128]
fp32 tensor written with one 4-descriptor DMA (per-partition-column
outputs would emit hundreds of 4-byte descriptors that crawl for >10us).
The host combines per-core partials exactly as relu/max commute.
"""

import numpy as np

B = 256
D = 1024
M = 8            # cores
BL = B // M      # 32 local rows per core
P = 128          # SBUF partitions
U = 2            # j = 2p + u column interleave
MARGIN = 0.2
NEG = -1.0e30    # diagonal mask value
NEG_INIT = -3.0e38

_NC = None


def _build_nc():
    import concourse.bacc as bacc
    from concourse import mybir
    from concourse.tile import TileContext

    f32 = mybir.dt.float32
    f16 = mybir.dt.float16
    add = mybir.AluOpType.add
    mult = mybir.AluOpType.mult
    amax = mybir.AluOpType.max

    nc = bacc.Bacc(None, target_bir_lowering=False, debug=False)
    im_d = nc.declare_dram_parameter("im", [B, D], f16, isOutput=False)
    s_d = nc.declare_dram_parameter("s", [BL, B, D], f16, isOutput=False)
    mt_d = nc.declare_dram_parameter("mask_t_neg", [P, U * BL], f32, isOutput=False)
    nr_d = nc.declare_dram_parameter("neg_rows", [BL, B], f32, isOutput=False)
    er_d = nc.declare_dram_parameter("eye_rows", [BL, B], f32, isOutput=False)
    o_d = nc.declare_dram_parameter("out", [4, P], f32, isOutput=True)

    with TileContext(nc) as tc:
        with (
            tc.tile_pool(name="const", bufs=1) as cpool,
            tc.tile_pool(name="sload", bufs=5) as spool,
            tc.tile_pool(name="scratch", bufs=2) as prpool,
            tc.tile_pool(name="prods", bufs=6) as mpool,
            tc.tile_pool(name="small", bufs=1) as smpool,
            tc.psum_pool(name="pgarbF", bufs=1) as gpoolF,
            tc.psum_pool(name="pgarbA", bufs=1) as gpoolA,
        ):
            # im packed as [p, u*D + d] with j = 2p + u, matching s tiles.
            # Rides the ACT HWDGE ring with the masks; s opens on Sync.
            im_t = cpool.tile([P, U * D], f16, tag="im")
            nc.scalar.dma_start(
                out=im_t[:].rearrange("p (u d) -> p u d", u=U),
                in_=im_d[:].rearrange("(p u) d -> p u d", p=P),
            )
            mt_t = cpool.tile([P, U * BL], f32, tag="maskT")
            nc.scalar.dma_start(out=mt_t[:], in_=mt_d[:])
            nr_t = cpool.tile([BL, B], f32, tag="negrows")
            nc.scalar.dma_start(out=nr_t[:], in_=nr_d[:])
            er_t = cpool.tile([BL, B], f32, tag="eyerows")
            nc.scalar.dma_start(out=er_t[:], in_=er_d[:])

            # scores^T: partition p, free column u*BL + i  (j = 2p + u).
            # One accumulator per engine: a shared tile would serialize
            # DVE and ACT into convoys (every accum write is ordered
            # against the previous engine's write to the same tile).
            # Both start at 0; each column is written by exactly one
            # engine; the epilogue merges with a single add.
            scoresD = smpool.tile([P, U * BL], f32, tag="scoresD")
            scoresA = smpool.tile([P, U * BL], f32, tag="scoresA")
            nc.gpsimd.memset(scoresD[:], 0.0)
            nc.gpsimd.memset(scoresA[:], 0.0)
            scoresT = smpool.tile([P, U * BL], f32, tag="scoresT")

            # Ramped chunks: tiny first so the first fused pass starts
            # ~1.5us in; tiny last so the stream tail is short.
            chunk_rows = [1, 1, 2, 4, 4, 4, 4, 4, 4, 2, 1, 1]
            assert sum(chunk_rows) == BL

            # Per-ROW engine assignment, measured on HW:
            #   F-row: 2x DVE fused scalar_tensor_tensor (1223 + 140 ns
            #      each, 1 elem/cyc ALU-bound; garbage out -> PSUM).
            #   A-row: ONE DVE fp16 mul [P, 2*D] -> fp16 SBUF (2 elem/cyc,
            #      930 ns; covers both halves), then 2x ACT accum
            #      (1148 + 283 ns each, write-back -> PSUM bank).
            # 12 F-rows / 20 A-rows balances DVE (~57 us) and ACT (~56 us),
            # just above the ~52 us fp16 HBM stream. First/last rows stay F.
            f_rows = {round(k * (BL - 1) / 11) for k in range(12)}
            assert len(f_rows) == 12 and 0 in f_rows and BL - 1 in f_rows

            # All s chunks ride the Sync HWDGE ring: the scalar ring's
            # DGE is the ACT engine, and ACT is ~95% busy accumulating —
            # chunks issued there arrive late and stall DVE.
            row0 = 0
            for ci, nr in enumerate(chunk_rows):
                s_t = spool.tile([P, nr * U * D], f16, tag="s")
                ring = nc.sync
                ring.dma_start(
                    out=s_t[:, 0:nr * U * D].rearrange(
                        "p (r f) -> p r f", r=nr
                    ),
                    in_=s_d[row0:row0 + nr].rearrange(
                        "r (p u) d -> p r (u d)", p=P
                    ),
                )
                for r in range(nr):
                    i = row0 + r
                    if i in f_rows:
                        for u in range(U):
                            off = (r * U + u) * D
                            garb = gpoolF.tile([P, D], f32, tag="garbF")
                            nc.vector.scalar_tensor_tensor(
                                out=garb[:],
                                in0=s_t[:, off:off + D],
                                scalar=1.0,
                                in1=im_t[:, u * D:(u + 1) * D],
                                op0=mult,
                                op1=mult,
                                accum_out=scoresD[:, u * BL + i:u * BL + i + 1],
                            )
                    else:
                        off = r * U * D
                        prod = mpool.tile([P, U * D], f16, tag="prod")
                        nc.vector.tensor_mul(
                            prod[:],
                            s_t[:, off:off + U * D],
                            im_t[:],
                        )
                        for u in range(U):
                            garbA = gpoolA.tile([P, D], f32, tag="garbA")
                            nc.scalar.activation(
                                out=garbA[:],
                                in_=prod[:, u * D:(u + 1) * D],
                                func=mybir.ActivationFunctionType.Copy,
                                accum_out=scoresA[:, u * BL + i:u * BL + i + 1],
                            )
                row0 += nr

            # Merge the two engine accumulators (disjoint columns, 0 else)
            nc.vector.tensor_add(scoresT[:], scoresD[:], scoresA[:])

            # Packed output tile: col 0/1 = colmax (u=0/1), col 2 = diag,
            # col 3 = rowcost. Transposed at the end into [4, 128]. The
            # memset covers the pad lanes the transposes read.
            out_t = smpool.tile([P, 32], f32, tag="out_t")
            nc.gpsimd.memset(out_t[:], 0.0)

            # Column maxima over local rows, diagonal masked to -1e30:
            # fused (scoresT + mask) then max-reduce.
            for u in range(U):
                cscr = prpool.tile([P, BL], f32, tag="cscr")
                nc.vector.tensor_add(
                    cscr[:],
                    scoresT[:, u * BL:(u + 1) * BL],
                    mt_t[:, u * BL:(u + 1) * BL],
                )
                nc.vector.reduce_max(
                    out_t[:, u:u + 1], cscr[:], axis=mybir.AxisListType.X
                )

            # Transpose scores^T -> rows [32, 256] via 32x32 stream blocks.
            # rows[i, u*128 + pp] = score(i, j=2*pp+u).
            rows = smpool.tile([BL, B], f32, tag="rows")
            for u in range(U):
                for k in range(P // 32):
                    nc.vector.transpose(
                        out=rows[0:BL, u * P + k * 32:u * P + (k + 1) * 32],
                        in_=scoresT[k * 32:(k + 1) * 32, u * BL:(u + 1) * BL],
                    )

            # rowmax (diag masked) and diag, both fused single passes.
            rowstat = smpool.tile([BL, 4], f32, tag="rowstat")
            rscr1 = prpool.tile([BL, B], f32, tag="rscr")
            nc.vector.tensor_add(rscr1[:], rows[:], nr_t[:])
            nc.vector.reduce_max(
                rowstat[:, 0:1], rscr1[:], axis=mybir.AxisListType.X
            )
            rscr2 = prpool.tile([BL, B], f32, tag="rscr")
            # diag = sum(rows * eye) fused in one pass
            nc.vector.scalar_tensor_tensor(
                out=rscr2[:],
                in0=rows[:],
                scalar=1.0,
                in1=er_t[:],
                op0=mult,
                op1=mult,
                accum_out=out_t[0:BL, 2:3],
            )
            # rowcost = relu(margin + rowmax - diag)
            nc.vector.tensor_sub(rowstat[:, 1:2], rowstat[:, 0:1], out_t[0:BL, 2:3])
            nc.vector.tensor_scalar(
                out=out_t[0:BL, 3:4], in0=rowstat[:, 1:2],
                scalar1=MARGIN, scalar2=0.0, op0=add, op1=amax,
            )

            # Pack: transpose out_t's first 4 columns into rows of outT,
            # then ONE 4-descriptor DMA (512B per partition line).
            outT = smpool.tile([32, P], f32, tag="outT")
            for k in range(P // 32):
                nc.vector.transpose(
                    out=outT[0:32, k * 32:(k + 1) * 32],
                    in_=out_t[k * 32:(k + 1) * 32, 0:32],
                )
            nc.scalar.dma_start(out=o_d[:], in_=outT[0:4, 0:P])

    nc.compile()
    return nc


def _get_nc():
    global _NC
    if _NC is None:
        _NC = _build_nc()
    return _NC


def _make_in_maps(im, s):
    im16 = im.astype(np.float16)
    s16 = s.astype(np.float16)
    il = np.arange(BL)
    # column q in `rows` layout: q = u*128 + pp  <->  j = 2*pp + u
    jq = 2 * (np.arange(B) % P) + (np.arange(B) // P)
    in_maps = []
    for c in range(M):
        jdiag = c * BL + il                      # global row index of local i
        mt = np.zeros((P, U * BL), np.float32)   # mt[p, u*BL+i]
        pd, ud = jdiag % P, jdiag // P
        # j = 2p+u == jdiag  =>  p = jdiag//2, u = jdiag%2
        mt[jdiag // 2, (jdiag % 2) * BL + il] = NEG
        nr = np.zeros((BL, B), np.float32)
        er = np.zeros((BL, B), np.float32)
        qdiag = (jdiag % 2) * P + jdiag // 2     # q with j(q) == jdiag
        nr[il, qdiag] = NEG
        er[il, qdiag] = 1.0
        in_maps.append({
            "im": im16,
            "s": s16[c * BL:(c + 1) * BL],
            "mask_t_neg": mt,
            "neg_rows": nr,
            "eye_rows": er,
        })
    return in_maps


def _combine(results):
    colmax = np.full(B, -np.inf, np.float32)
    rowcosts = np.empty(B, np.float32)
    diag = np.empty(B, np.float32)
    for c in range(M):
        o = results[c]["out"]                    # [4, 128] fp32
        cm = np.stack([o[0], o[1]], axis=1).ravel()   # j = 2p+u
        colmax = np.maximum(colmax, cm)
        diag[c * BL:(c + 1) * BL] = o[2, :BL]
        rowcosts[c * BL:(c + 1) * BL] = o[3, :BL]
    cost_im = np.maximum(np.float32(MARGIN) + colmax - diag, np.float32(0.0))
    loss = rowcosts.sum(dtype=np.float32) + cost_im.sum(dtype=np.float32)
    return np.array(loss, dtype=np.float32)


def _run(im, s, **spmd_kwargs):
    from concourse.bass_utils import run_bass_kernel_spmd

    im = np.ascontiguousarray(np.asarray(im), dtype=np.float32)
    s = np.ascontiguousarray(np.asarray(s), dtype=np.float32)
    nc = _get_nc()
    res = run_bass_kernel_spmd(nc, _make_in_maps(im, s), list(range(M)),
                               **spmd_kwargs)
    return _combine(res.results), res


def kernel(im, s):
    loss, _ = _run(im, s)
    return loss


# revision 26
# speedup vs baseline: 1.2438x; 1.1208x over previous
"""Trainium2 Bass kernel for nn_ContrastiveLoss3DTo2D.

Reference computation (B=256, D=1024, margin=0.2):
    scores[i, j] = dot(im[j], s[i, j])                    # [B, B]
    cost_s  = sum_i relu(margin + max_{j!=i} scores[i,j] - scores[i,i])
    cost_im = sum_j relu(margin + max_{i!=j} scores[i,j] - scores[j,j])
    loss = cost_s + cost_im

Sharding: s (and the score matrix) is sharded along i across 8 cores
(32 rows each); im is replicated. Inputs are cast to fp16 on the host
(loss tolerance is 2e-2; fp16 keeps the dot-product error ~1e-4 rel)
which halves HBM traffic — the binding constraint — and doubles DVE
elementwise throughput. Each core streams its 16 MB shard and computes
the 32x256 score block with ONE fused DVE pass per (row, half):
tensor_tensor_reduce does multiply + free-axis accumulate (fp32) in a
single instruction, leaving ACT/PE idle.

Column layout: j = 2p + u (partition p in [0,128), u in {0,1}) so each
DMA descriptor is a contiguous 4 KB run (two adjacent j rows of D).
Per-core reductions produce colmax[256] (diag masked), diag[32], and
rowcost[32], packed via 32x32 stream transposes into a single [4,128]
fp32 tensor written with one 4-descriptor DMA (per-partition-column
outputs would emit hundreds of 4-byte descriptors that crawl for >10us).
The host combines per-core partials exactly as relu/max commute.
"""

import numpy as np

B = 256
D = 1024
M = 8            # cores
BL = B // M      # 32 local rows per core
P = 128          # SBUF partitions
U = 2            # j = 2p + u column interleave
MARGIN = 0.2
NEG = -1.0e30    # diagonal mask value
NEG_INIT = -3.0e38

_NC = None


def _build_nc():
    import concourse.bacc as bacc
    from concourse import mybir
    from concourse.tile import TileContext

    f32 = mybir.dt.float32
    f16 = mybir.dt.float16
    add = mybir.AluOpType.add
    mult = mybir.AluOpType.mult
    amax = mybir.AluOpType.max

    nc = bacc.Bacc(None, target_bir_lowering=False, debug=False)
    im_d = nc.declare_dram_parameter("im", [B, D], f16, isOutput=False)
    s_d = nc.declare_dram_parameter("s", [BL, B, D], f16, isOutput=False)
    mt_d = nc.declare_dram_parameter("mask_t_neg", [P, U * BL], f32, isOutput=False)
    nr_d = nc.declare_dram_parameter("neg_rows", [BL, B], f32, isOutput=False)
    er_d = nc.declare_dram_parameter("eye_rows", [BL, B], f32, isOutput=False)
    o_d = nc.declare_dram_parameter("out", [4, P], f32, isOutput=True)

    with TileContext(nc) as tc:
        with (
            tc.tile_pool(name="const", bufs=1) as cpool,
            tc.tile_pool(name="sload", bufs=5) as spool,
            tc.tile_pool(name="scratch", bufs=2) as prpool,
            tc.tile_pool(name="prods", bufs=6) as mpool,
            tc.tile_pool(name="small", bufs=1) as smpool,
            tc.psum_pool(name="pgarbF", bufs=1) as gpoolF,
            tc.psum_pool(name="pgarbA", bufs=1) as gpoolA,
        ):
            # im packed as [p, u*D + d] with j = 2p + u, matching s tiles.
            # First on the Sync ring so compute can start ~11us in; the
            # epilogue-only masks ride the scalar ring (ACT is idle early).
            im_t = cpool.tile([P, U * D], f16, tag="im")
            nc.sync.dma_start(
                out=im_t[:].rearrange("p (u d) -> p u d", u=U),
                in_=im_d[:].rearrange("(p u) d -> p u d", p=P),
            )
            mt_t = cpool.tile([P, U * BL], f32, tag="maskT")
            nc.scalar.dma_start(out=mt_t[:], in_=mt_d[:])
            nr_t = cpool.tile([BL, B], f32, tag="negrows")
            nc.scalar.dma_start(out=nr_t[:], in_=nr_d[:])
            er_t = cpool.tile([BL, B], f32, tag="eyerows")
            nc.scalar.dma_start(out=er_t[:], in_=er_d[:])

            # scores^T: partition p, free column u*BL + i  (j = 2p + u).
            # One accumulator per engine: a shared tile would serialize
            # DVE and ACT into convoys (every accum write is ordered
            # against the previous engine's write to the same tile).
            # Both start at 0; each column is written by exactly one
            # engine; the epilogue merges with a single add.
            scoresD = smpool.tile([P, U * BL], f32, tag="scoresD")
            scoresA = smpool.tile([P, U * BL], f32, tag="scoresA")
            nc.gpsimd.memset(scoresD[:], 0.0)
            nc.gpsimd.memset(scoresA[:], 0.0)
            scoresT = smpool.tile([P, U * BL], f32, tag="scoresT")

            # Ramped chunks: tiny first so the first fused pass starts
            # ~1.5us in; tiny last so the stream tail is short.
            chunk_rows = [1, 1, 2, 4, 4, 4, 4, 4, 4, 2, 1, 1]
            assert sum(chunk_rows) == BL

            # Per-ROW engine assignment, measured on HW:
            #   F-row: 2x DVE fused scalar_tensor_tensor (1223 + 140 ns
            #      each, 1 elem/cyc ALU-bound; garbage out -> PSUM).
            #   A-row: ONE DVE fp16 mul [P, 2*D] -> fp16 SBUF (2 elem/cyc,
            #      930 ns; covers both halves), then 2x ACT accum
            #      (1148 + 283 ns each, write-back -> PSUM bank).
            # 11 F-rows / 21 A-rows balances DVE and ACT at ~70 us under
            # the measured engine-overlap contention (ops run ~18% slower
            # when DVE/ACT/DMA all stream SBUF). First/last rows stay F.
            f_rows = {round(k * (BL - 1) / 10) for k in range(11)}
            assert len(f_rows) == 11 and 0 in f_rows and BL - 1 in f_rows

            # All s chunks ride the Sync HWDGE ring: the scalar ring's
            # DGE is the ACT engine, and ACT is ~95% busy accumulating —
            # chunks issued there arrive late and stall DVE.
            row0 = 0
            for ci, nr in enumerate(chunk_rows):
                s_t = spool.tile([P, nr * U * D], f16, tag="s")
                ring = nc.sync
                ring.dma_start(
                    out=s_t[:, 0:nr * U * D].rearrange(
                        "p (r f) -> p r f", r=nr
                    ),
                    in_=s_d[row0:row0 + nr].rearrange(
                        "r (p u) d -> p r (u d)", p=P
                    ),
                )
                for r in range(nr):
                    i = row0 + r
                    if i in f_rows:
                        for u in range(U):
                            off = (r * U + u) * D
                            garb = gpoolF.tile([P, D], f32, tag="garbF")
                            nc.vector.scalar_tensor_tensor(
                                out=garb[:],
                                in0=s_t[:, off:off + D],
                                scalar=1.0,
                                in1=im_t[:, u * D:(u + 1) * D],
                                op0=mult,
                                op1=mult,
                                accum_out=scoresD[:, u * BL + i:u * BL + i + 1],
                            )
                    else:
                        off = r * U * D
                        prod = mpool.tile([P, U * D], f16, tag="prod")
                        nc.vector.tensor_mul(
                            prod[:],
                            s_t[:, off:off + U * D],
                            im_t[:],
                        )
                        for u in range(U):
                            garbA = gpoolA.tile([P, D], f32, tag="garbA")
                            nc.scalar.activation(
                                out=garbA[:],
                                in_=prod[:, u * D:(u + 1) * D],
                                func=mybir.ActivationFunctionType.Copy,
                                accum_out=scoresA[:, u * BL + i:u * BL + i + 1],
                            )
                row0 += nr

            # Merge the two engine accumulators (disjoint columns, 0 else)
            nc.vector.tensor_add(scoresT[:], scoresD[:], scoresA[:])

            # Packed output tile: col 0/1 = colmax (u=0/1), col 2 = diag,
            # col 3 = rowcost. Transposed at the end into [4, 128]. The
            # memset covers the pad lanes the transposes read.
            out_t = smpool.tile([P, 32], f32, tag="out_t")
            nc.gpsimd.memset(out_t[:], 0.0)

            # Column maxima over local rows, diagonal masked to -1e30:
            # fused (scoresT + mask) then max-reduce.
            for u in range(U):
                cscr = prpool.tile([P, BL], f32, tag="cscr")
                nc.vector.tensor_add(
                    cscr[:],
                    scoresT[:, u * BL:(u + 1) * BL],
                    mt_t[:, u * BL:(u + 1) * BL],
                )
                nc.vector.reduce_max(
                    out_t[:, u:u + 1], cscr[:], axis=mybir.AxisListType.X
                )

            # Transpose scores^T -> rows [32, 256] via 32x32 stream blocks.
            # rows[i, u*128 + pp] = score(i, j=2*pp+u).
            rows = smpool.tile([BL, B], f32, tag="rows")
            for u in range(U):
                for k in range(P // 32):
                    nc.vector.transpose(
                        out=rows[0:BL, u * P + k * 32:u * P + (k + 1) * 32],
                        in_=scoresT[k * 32:(k + 1) * 32, u * BL:(u + 1) * BL],
                    )

            # rowmax (diag masked) and diag, both fused single passes.
            rowstat = smpool.tile([BL, 4], f32, tag="rowstat")
            rscr1 = prpool.tile([BL, B], f32, tag="rscr")
            nc.vector.tensor_add(rscr1[:], rows[:], nr_t[:])
            nc.vector.reduce_max(
                rowstat[:, 0:1], rscr1[:], axis=mybir.AxisListType.X
            )
            rscr2 = prpool.tile([BL, B], f32, tag="rscr")
            # diag = sum(rows * eye) fused in one pass
            nc.vector.scalar_tensor_tensor(
                out=rscr2[:],
                in0=rows[:],
                scalar=1.0,
                in1=er_t[:],
                op0=mult,
                op1=mult,
                accum_out=out_t[0:BL, 2:3],
            )
            # rowcost = relu(margin + rowmax - diag)
            nc.vector.tensor_sub(rowstat[:, 1:2], rowstat[:, 0:1], out_t[0:BL, 2:3])
            nc.vector.tensor_scalar(
                out=out_t[0:BL, 3:4], in0=rowstat[:, 1:2],
                scalar1=MARGIN, scalar2=0.0, op0=add, op1=amax,
            )

            # Pack: transpose out_t's first 4 columns into rows of outT,
            # then ONE 4-descriptor DMA (512B per partition line).
            outT = smpool.tile([32, P], f32, tag="outT")
            for k in range(P // 32):
                nc.vector.transpose(
                    out=outT[0:32, k * 32:(k + 1) * 32],
                    in_=out_t[k * 32:(k + 1) * 32, 0:32],
                )
            nc.scalar.dma_start(out=o_d[:], in_=outT[0:4, 0:P])

    nc.compile()
    return nc


def _get_nc():
    global _NC
    if _NC is None:
        _NC = _build_nc()
    return _NC


def _make_in_maps(im, s):
    im16 = im.astype(np.float16)
    s16 = s.astype(np.float16)
    il = np.arange(BL)
    # column q in `rows` layout: q = u*128 + pp  <->  j = 2*pp + u
    jq = 2 * (np.arange(B) % P) + (np.arange(B) // P)
    in_maps = []
    for c in range(M):
        jdiag = c * BL + il                      # global row index of local i
        mt = np.zeros((P, U * BL), np.float32)   # mt[p, u*BL+i]
        pd, ud = jdiag % P, jdiag // P
        # j = 2p+u == jdiag  =>  p = jdiag//2, u = jdiag%2
        mt[jdiag // 2, (jdiag % 2) * BL + il] = NEG
        nr = np.zeros((BL, B), np.float32)
        er = np.zeros((BL, B), np.float32)
        qdiag = (jdiag % 2) * P + jdiag // 2     # q with j(q) == jdiag
        nr[il, qdiag] = NEG
        er[il, qdiag] = 1.0
        in_maps.append({
            "im": im16,
            "s": s16[c * BL:(c + 1) * BL],
            "mask_t_neg": mt,
            "neg_rows": nr,
            "eye_rows": er,
        })
    return in_maps


def _combine(results):
    colmax = np.full(B, -np.inf, np.float32)
    rowcosts = np.empty(B, np.float32)
    diag = np.empty(B, np.float32)
    for c in range(M):
        o = results[c]["out"]                    # [4, 128] fp32
        cm = np.stack([o[0], o[1]], axis=1).ravel()   # j = 2p+u
        colmax = np.maximum(colmax, cm)
        diag[c * BL:(c + 1) * BL] = o[2, :BL]
        rowcosts[c * BL:(c + 1) * BL] = o[3, :BL]
    cost_im = np.maximum(np.float32(MARGIN) + colmax - diag, np.float32(0.0))
    loss = rowcosts.sum(dtype=np.float32) + cost_im.sum(dtype=np.float32)
    return np.array(loss, dtype=np.float32)


def _run(im, s, **spmd_kwargs):
    from concourse.bass_utils import run_bass_kernel_spmd

    im = np.ascontiguousarray(np.asarray(im), dtype=np.float32)
    s = np.ascontiguousarray(np.asarray(s), dtype=np.float32)
    nc = _get_nc()
    res = run_bass_kernel_spmd(nc, _make_in_maps(im, s), list(range(M)),
                               **spmd_kwargs)
    return _combine(res.results), res


def kernel(im, s):
    loss, _ = _run(im, s)
    return loss


# revision 33
# speedup vs baseline: 1.2875x; 1.0351x over previous
"""Trainium2 Bass kernel for nn_ContrastiveLoss3DTo2D.

Reference computation (B=256, D=1024, margin=0.2):
    scores[i, j] = dot(im[j], s[i, j])                    # [B, B]
    cost_s  = sum_i relu(margin + max_{j!=i} scores[i,j] - scores[i,i])
    cost_im = sum_j relu(margin + max_{i!=j} scores[i,j] - scores[j,j])
    loss = cost_s + cost_im

Sharding: s (and the score matrix) is sharded along i across 8 cores
(32 rows each); im is replicated. Inputs are cast to fp16 on the host
(loss tolerance is 2e-2; fp16 keeps the dot-product error ~1e-4 rel)
which halves HBM traffic — the binding constraint — and doubles DVE
elementwise throughput. Each core streams its 16 MB shard and computes
the 32x256 score block with ONE fused DVE pass per (row, half):
tensor_tensor_reduce does multiply + free-axis accumulate (fp32) in a
single instruction, leaving ACT/PE idle.

Column layout: j = 2p + u (partition p in [0,128), u in {0,1}) so each
DMA descriptor is a contiguous 4 KB run (two adjacent j rows of D).
Per-core reductions produce colmax[256] (diag masked), diag[32], and
rowcost[32], packed via 32x32 stream transposes into a single [4,128]
fp32 tensor written with one 4-descriptor DMA (per-partition-column
outputs would emit hundreds of 4-byte descriptors that crawl for >10us).
The host combines per-core partials exactly as relu/max commute.
"""

import numpy as np

B = 256
D = 1024
M = 8            # cores
BL = B // M      # 32 local rows per core
P = 128          # SBUF partitions
U = 2            # j = 2p + u column interleave
MARGIN = 0.2
NEG = -1.0e30    # diagonal mask value
NEG_INIT = -3.0e38

_NC = None


def _build_nc():
    import concourse.bacc as bacc
    from concourse import mybir
    from concourse.tile import TileContext

    f32 = mybir.dt.float32
    f16 = mybir.dt.float16
    add = mybir.AluOpType.add
    mult = mybir.AluOpType.mult
    amax = mybir.AluOpType.max

    nc = bacc.Bacc(None, target_bir_lowering=False, debug=False)
    im_d = nc.declare_dram_parameter("im", [B, D], f16, isOutput=False)
    s_d = nc.declare_dram_parameter("s", [BL, B, D], f16, isOutput=False)
    mt_d = nc.declare_dram_parameter("mask_t_neg", [P, U * BL], f32, isOutput=False)
    nr_d = nc.declare_dram_parameter("neg_rows", [BL, B], f32, isOutput=False)
    er_d = nc.declare_dram_parameter("eye_rows", [BL, B], f32, isOutput=False)
    o_d = nc.declare_dram_parameter("out", [4, P], f32, isOutput=True)

    with TileContext(nc) as tc:
        with (
            tc.tile_pool(name="const", bufs=1) as cpool,
            tc.tile_pool(name="sload", bufs=5) as spool,
            tc.tile_pool(name="scratch", bufs=2) as prpool,
            tc.tile_pool(name="prods", bufs=8) as mpool,
            tc.tile_pool(name="small", bufs=1) as smpool,
            tc.psum_pool(name="pgarbF", bufs=1) as gpoolF,
            tc.psum_pool(name="pgarbA", bufs=1) as gpoolA,
        ):
            # im packed as [p, u*D + d] with j = 2p + u, matching s tiles.
            # First on the Sync ring so compute can start ~11us in; the
            # epilogue-only masks ride the scalar ring (ACT is idle early).
            im_t = cpool.tile([P, U * D], f16, tag="im")
            nc.sync.dma_start(
                out=im_t[:].rearrange("p (u d) -> p u d", u=U),
                in_=im_d[:].rearrange("(p u) d -> p u d", p=P),
            )
            mt_t = cpool.tile([P, U * BL], f32, tag="maskT")
            nc.scalar.dma_start(out=mt_t[:], in_=mt_d[:])
            nr_t = cpool.tile([BL, B], f32, tag="negrows")
            nc.scalar.dma_start(out=nr_t[:], in_=nr_d[:])
            er_t = cpool.tile([BL, B], f32, tag="eyerows")
            nc.scalar.dma_start(out=er_t[:], in_=er_d[:])

            # scores^T: partition p, free column u*BL + i  (j = 2p + u).
            # One accumulator per engine: a shared tile would serialize
            # DVE and ACT into convoys (every accum write is ordered
            # against the previous engine's write to the same tile).
            # Both start at 0; each column is written by exactly one
            # engine; the epilogue merges with a single add.
            scoresD = smpool.tile([P, U * BL], f32, tag="scoresD")
            scoresA = smpool.tile([P, U * BL], f32, tag="scoresA")
            scoresG = smpool.tile([P, U * BL], f32, tag="scoresG")
            nc.gpsimd.memset(scoresD[:], 0.0)
            nc.gpsimd.memset(scoresA[:], 0.0)
            nc.gpsimd.memset(scoresG[:], 0.0)
            scoresT = smpool.tile([P, U * BL], f32, tag="scoresT")

            # Ramped chunks: tiny first so the first fused pass starts
            # ~1.5us in; tiny last so the stream tail is short. The
            # front-loaded ramp keeps DMA ahead of DVE's early pace.
            chunk_rows = [1, 2, 3, 4, 4, 4, 4, 4, 2, 2, 1, 1]
            assert sum(chunk_rows) == BL

            # Per-ROW engine assignment, measured on HW:
            #   F-row: 2x DVE fused scalar_tensor_tensor (1223 + 140 ns
            #      each, 1 elem/cyc ALU-bound; garbage out -> PSUM).
            #   A-row: ONE DVE fp16 mul [P, 2*D] -> fp16 SBUF (2 elem/cyc,
            #      930 ns; covers both halves), then 2x ACT accum
            #      (1148 + 283 ns each, write-back -> PSUM bank).
            # 11 F-rows / 21 A-rows balances DVE and ACT at ~58 us each
            # (GpSimd can't help: its tensor_reduce is partition-axis
            # only, and its big muls poison DVE's 2-ports mode).
            # First/last rows stay F.
            f_rows = {round(k * (BL - 1) / 10) for k in range(11)}
            assert len(f_rows) == 11 and 0 in f_rows and BL - 1 in f_rows
            g_rows = set()

            # All s chunks ride the Sync HWDGE ring: the scalar ring's
            # DGE is the ACT engine, and ACT is ~95% busy accumulating —
            # chunks issued there arrive late and stall DVE.
            row0 = 0
            for ci, nr in enumerate(chunk_rows):
                s_t = spool.tile([P, nr * U * D], f16, tag="s")
                ring = nc.sync
                ring.dma_start(
                    out=s_t[:, 0:nr * U * D].rearrange(
                        "p (r f) -> p r f", r=nr
                    ),
                    in_=s_d[row0:row0 + nr].rearrange(
                        "r (p u) d -> p r (u d)", p=P
                    ),
                )
                for r in range(nr):
                    i = row0 + r
                    if i in f_rows:
                        for u in range(U):
                            off = (r * U + u) * D
                            garb = gpoolF.tile([P, D], f32, tag="garbF")
                            nc.vector.scalar_tensor_tensor(
                                out=garb[:],
                                in0=s_t[:, off:off + D],
                                scalar=1.0,
                                in1=im_t[:, u * D:(u + 1) * D],
                                op0=mult,
                                op1=mult,
                                accum_out=scoresD[:, u * BL + i:u * BL + i + 1],
                            )
                    else:
                        off = r * U * D
                        prod = mpool.tile([P, U * D], f16, tag="prod")
                        nc.vector.tensor_mul(
                            prod[:],
                            s_t[:, off:off + U * D],
                            im_t[:],
                        )
                        if i in g_rows:
                            for u in range(U):
                                nc.gpsimd.reduce_sum(
                                    scoresG[:, u * BL + i:u * BL + i + 1],
                                    prod[:, u * D:(u + 1) * D],
                                    axis=mybir.AxisListType.X,
                                )
                        else:
                            for u in range(U):
                                garbA = gpoolA.tile([P, D], f32, tag="garbA")
                                nc.scalar.activation(
                                    out=garbA[:],
                                    in_=prod[:, u * D:(u + 1) * D],
                                    func=mybir.ActivationFunctionType.Copy,
                                    accum_out=scoresA[:, u * BL + i:u * BL + i + 1],
                                )
                row0 += nr

            # Merge the engine accumulators (disjoint columns, 0 elsewhere)
            nc.vector.tensor_add(scoresD[:], scoresD[:], scoresG[:])
            nc.vector.tensor_add(scoresT[:], scoresD[:], scoresA[:])

            # Packed output tile: col 0/1 = colmax (u=0/1), col 2 = diag,
            # col 3 = rowcost. Transposed at the end into [4, 128]. The
            # memset covers the pad lanes the transposes read.
            out_t = smpool.tile([P, 32], f32, tag="out_t")
            nc.gpsimd.memset(out_t[:], 0.0)

            # Column maxima over local rows, diagonal masked to -1e30:
            # fused (scoresT + mask) then max-reduce.
            for u in range(U):
                cscr = prpool.tile([P, BL], f32, tag="cscr")
                nc.vector.tensor_add(
                    cscr[:],
                    scoresT[:, u * BL:(u + 1) * BL],
                    mt_t[:, u * BL:(u + 1) * BL],
                )
                nc.vector.reduce_max(
                    out_t[:, u:u + 1], cscr[:], axis=mybir.AxisListType.X
                )

            # Transpose scores^T -> rows [32, 256] via 32x32 stream blocks.
            # rows[i, u*128 + pp] = score(i, j=2*pp+u).
            rows = smpool.tile([BL, B], f32, tag="rows")
            for u in range(U):
                for k in range(P // 32):
                    nc.vector.transpose(
                        out=rows[0:BL, u * P + k * 32:u * P + (k + 1) * 32],
                        in_=scoresT[k * 32:(k + 1) * 32, u * BL:(u + 1) * BL],
                    )

            # rowmax (diag masked) and diag, both fused single passes.
            rowstat = smpool.tile([BL, 4], f32, tag="rowstat")
            rscr1 = prpool.tile([BL, B], f32, tag="rscr")
            nc.vector.tensor_add(rscr1[:], rows[:], nr_t[:])
            nc.vector.reduce_max(
                rowstat[:, 0:1], rscr1[:], axis=mybir.AxisListType.X
            )
            rscr2 = prpool.tile([BL, B], f32, tag="rscr")
            # diag = sum(rows * eye) fused in one pass
            nc.vector.scalar_tensor_tensor(
                out=rscr2[:],
                in0=rows[:],
                scalar=1.0,
                in1=er_t[:],
                op0=mult,
                op1=mult,
                accum_out=out_t[0:BL, 2:3],
            )
            # rowcost = relu(margin + rowmax - diag)
            nc.vector.tensor_sub(rowstat[:, 1:2], rowstat[:, 0:1], out_t[0:BL, 2:3])
            nc.vector.tensor_scalar(
                out=out_t[0:BL, 3:4], in0=rowstat[:, 1:2],
                scalar1=MARGIN, scalar2=0.0, op0=add, op1=amax,
            )

            # Pack: transpose out_t's first 4 columns into rows of outT,
            # then ONE 4-descriptor DMA (512B per partition line).
            outT = smpool.tile([32, P], f32, tag="outT")
            for k in range(P // 32):
                nc.vector.transpose(
                    out=outT[0:32, k * 32:(k + 1) * 32],
                    in_=out_t[k * 32:(k + 1) * 32, 0:32],
                )
            nc.scalar.dma_start(out=o_d[:], in_=outT[0:4, 0:P])

    nc.compile()
    return nc


def _get_nc():
    global _NC
    if _NC is None:
        _NC = _build_nc()
    return _NC


def _make_in_maps(im, s):
    im16 = im.astype(np.float16)
    s16 = s.astype(np.float16)
    il = np.arange(BL)
    # column q in `rows` layout: q = u*128 + pp  <->  j = 2*pp + u
    jq = 2 * (np.arange(B) % P) + (np.arange(B) // P)
    in_maps = []
    for c in range(M):
        jdiag = c * BL + il                      # global row index of local i
        mt = np.zeros((P, U * BL), np.float32)   # mt[p, u*BL+i]
        pd, ud = jdiag % P, jdiag // P
        # j = 2p+u == jdiag  =>  p = jdiag//2, u = jdiag%2
        mt[jdiag // 2, (jdiag % 2) * BL + il] = NEG
        nr = np.zeros((BL, B), np.float32)
        er = np.zeros((BL, B), np.float32)
        qdiag = (jdiag % 2) * P + jdiag // 2     # q with j(q) == jdiag
        nr[il, qdiag] = NEG
        er[il, qdiag] = 1.0
        in_maps.append({
            "im": im16,
            "s": s16[c * BL:(c + 1) * BL],
            "mask_t_neg": mt,
            "neg_rows": nr,
            "eye_rows": er,
        })
    return in_maps


def _combine(results):
    colmax = np.full(B, -np.inf, np.float32)
    rowcosts = np.empty(B, np.float32)
    diag = np.empty(B, np.float32)
    for c in range(M):
        o = results[c]["out"]                    # [4, 128] fp32
        cm = np.stack([o[0], o[1]], axis=1).ravel()   # j = 2p+u
        colmax = np.maximum(colmax, cm)
        diag[c * BL:(c + 1) * BL] = o[2, :BL]
        rowcosts[c * BL:(c + 1) * BL] = o[3, :BL]
    cost_im = np.maximum(np.float32(MARGIN) + colmax - diag, np.float32(0.0))
    loss = rowcosts.sum(dtype=np.float32) + cost_im.sum(dtype=np.float32)
    return np.array(loss, dtype=np.float32)


def _run(im, s, **spmd_kwargs):
    from concourse.bass_utils import run_bass_kernel_spmd

    im = np.ascontiguousarray(np.asarray(im), dtype=np.float32)
    s = np.ascontiguousarray(np.asarray(s), dtype=np.float32)
    nc = _get_nc()
    res = run_bass_kernel_spmd(nc, _make_in_maps(im, s), list(range(M)),
                               **spmd_kwargs)
    return _combine(res.results), res


def kernel(im, s):
    loss, _ = _run(im, s)
    return loss


# revision 35
# speedup vs baseline: 1.3385x; 1.0396x over previous
"""Trainium2 Bass kernel for nn_ContrastiveLoss3DTo2D.

Reference computation (B=256, D=1024, margin=0.2):
    scores[i, j] = dot(im[j], s[i, j])                    # [B, B]
    cost_s  = sum_i relu(margin + max_{j!=i} scores[i,j] - scores[i,i])
    cost_im = sum_j relu(margin + max_{i!=j} scores[i,j] - scores[j,j])
    loss = cost_s + cost_im

Sharding: s (and the score matrix) is sharded along i across 8 cores
(32 rows each); im is replicated. Inputs are cast to fp16 on the host
(loss tolerance is 2e-2; fp16 keeps the dot-product error ~1e-4 rel)
which halves HBM traffic — the binding constraint — and doubles DVE
elementwise throughput. Each core streams its 16 MB shard and computes
the 32x256 score block with ONE fused DVE pass per (row, half):
tensor_tensor_reduce does multiply + free-axis accumulate (fp32) in a
single instruction, leaving ACT/PE idle.

Column layout: j = 2p + u (partition p in [0,128), u in {0,1}) so each
DMA descriptor is a contiguous 4 KB run (two adjacent j rows of D).
Per-core reductions produce colmax[256] (diag masked), diag[32], and
rowcost[32], packed via 32x32 stream transposes into a single [4,128]
fp32 tensor written with one 4-descriptor DMA (per-partition-column
outputs would emit hundreds of 4-byte descriptors that crawl for >10us).
The host combines per-core partials exactly as relu/max commute.
"""

import numpy as np

B = 256
D = 1024
M = 8            # cores
BL = B // M      # 32 local rows per core
P = 128          # SBUF partitions
U = 2            # j = 2p + u column interleave
MARGIN = 0.2
NEG = -1.0e30    # diagonal mask value
NEG_INIT = -3.0e38

_NC = None


def _build_nc():
    import concourse.bacc as bacc
    from concourse import mybir
    from concourse.tile import TileContext

    f32 = mybir.dt.float32
    f16 = mybir.dt.float16
    add = mybir.AluOpType.add
    mult = mybir.AluOpType.mult
    amax = mybir.AluOpType.max

    nc = bacc.Bacc(None, target_bir_lowering=False, debug=False)
    im_d = nc.declare_dram_parameter("im", [B, D], f16, isOutput=False)
    s_d = nc.declare_dram_parameter("s", [BL, B, D], f16, isOutput=False)
    mt_d = nc.declare_dram_parameter("mask_t_neg", [P, U * BL], f32, isOutput=False)
    nr_d = nc.declare_dram_parameter("neg_rows", [BL, B], f32, isOutput=False)
    er_d = nc.declare_dram_parameter("eye_rows", [BL, B], f32, isOutput=False)
    o_d = nc.declare_dram_parameter("out", [4, P], f32, isOutput=True)

    with TileContext(nc) as tc:
        with (
            tc.tile_pool(name="const", bufs=1) as cpool,
            tc.tile_pool(name="sload", bufs=5) as spool,
            tc.tile_pool(name="scratch", bufs=2) as prpool,
            tc.tile_pool(name="prods", bufs=12) as mpool,
            tc.tile_pool(name="small", bufs=1) as smpool,
            tc.psum_pool(name="pgarbF", bufs=1) as gpoolF,
            tc.psum_pool(name="pgarbA", bufs=1) as gpoolA,
        ):
            # im packed as [p, u*D + d] with j = 2p + u, matching s tiles.
            # First on the Sync ring so compute can start ~11us in; the
            # epilogue-only masks ride the scalar ring (ACT is idle early).
            im_t = cpool.tile([P, U * D], f16, tag="im")
            nc.sync.dma_start(
                out=im_t[:].rearrange("p (u d) -> p u d", u=U),
                in_=im_d[:].rearrange("(p u) d -> p u d", p=P),
            )
            mt_t = cpool.tile([P, U * BL], f32, tag="maskT")
            nc.scalar.dma_start(out=mt_t[:], in_=mt_d[:])
            nr_t = cpool.tile([BL, B], f32, tag="negrows")
            nc.scalar.dma_start(out=nr_t[:], in_=nr_d[:])
            er_t = cpool.tile([BL, B], f32, tag="eyerows")
            nc.scalar.dma_start(out=er_t[:], in_=er_d[:])

            # scores^T: partition p, free column u*BL + i  (j = 2p + u).
            # One accumulator per engine: a shared tile would serialize
            # DVE and ACT into convoys (every accum write is ordered
            # against the previous engine's write to the same tile).
            # Both start at 0; each column is written by exactly one
            # engine; the epilogue merges with a single add.
            scoresD = smpool.tile([P, U * BL], f32, tag="scoresD")
            scoresA = smpool.tile([P, U * BL], f32, tag="scoresA")
            nc.gpsimd.memset(scoresD[:], 0.0)
            nc.gpsimd.memset(scoresA[:], 0.0)
            scoresT = smpool.tile([P, U * BL], f32, tag="scoresT")

            # Ramped chunks: tiny first so the first fused pass starts
            # ~1.5us in; tiny last so the stream tail is short. The
            # front-loaded ramp keeps DMA ahead of DVE's early pace.
            chunk_rows = [1, 2, 3, 4, 4, 4, 4, 4, 2, 2, 1, 1]
            assert sum(chunk_rows) == BL

            # Per-ROW engine assignment, measured on HW:
            #   F-row: 2x DVE fused scalar_tensor_tensor (1223 + 140 ns
            #      each, 1 elem/cyc ALU-bound; garbage out -> PSUM).
            #   A-row: ONE DVE fp16 mul [P, 2*D] -> fp16 SBUF (2 elem/cyc,
            #      930 ns; covers both halves), then 2x ACT accum
            #      (1148 + 283 ns each, write-back -> PSUM bank).
            # 13 F-rows / 19 A-rows, with F clustered at the END: once
            # the stream is done DVE self-accumulates the last rows while
            # ACT drains its backlog of earlier A-rows, instead of DVE
            # idling behind ACT. (GpSimd can't help: its tensor_reduce
            # is partition-axis only, and its big muls poison DVE's
            # 2-ports mode.)
            f_rows = {0, 3, 6, 9, 12, 15, 18, 21, 24, 28, 29, 30, 31}
            assert 0 in f_rows and BL - 1 in f_rows

            # All s chunks ride the Sync HWDGE ring: the scalar ring's
            # DGE is the ACT engine, and ACT is ~95% busy accumulating —
            # chunks issued there arrive late and stall DVE.
            row0 = 0
            for ci, nr in enumerate(chunk_rows):
                s_t = spool.tile([P, nr * U * D], f16, tag="s")
                ring = nc.sync
                ring.dma_start(
                    out=s_t[:, 0:nr * U * D].rearrange(
                        "p (r f) -> p r f", r=nr
                    ),
                    in_=s_d[row0:row0 + nr].rearrange(
                        "r (p u) d -> p r (u d)", p=P
                    ),
                )
                for r in range(nr):
                    i = row0 + r
                    if i in f_rows:
                        for u in range(U):
                            off = (r * U + u) * D
                            garb = gpoolF.tile([P, D], f32, tag="garbF")
                            nc.vector.scalar_tensor_tensor(
                                out=garb[:],
                                in0=s_t[:, off:off + D],
                                scalar=1.0,
                                in1=im_t[:, u * D:(u + 1) * D],
                                op0=mult,
                                op1=mult,
                                accum_out=scoresD[:, u * BL + i:u * BL + i + 1],
                            )
                    else:
                        off = r * U * D
                        prod = mpool.tile([P, U * D], f16, tag="prod")
                        nc.vector.tensor_mul(
                            prod[:],
                            s_t[:, off:off + U * D],
                            im_t[:],
                        )
                        for u in range(U):
                            garbA = gpoolA.tile([P, D], f32, tag="garbA")
                            nc.scalar.activation(
                                out=garbA[:],
                                in_=prod[:, u * D:(u + 1) * D],
                                func=mybir.ActivationFunctionType.Copy,
                                accum_out=scoresA[:, u * BL + i:u * BL + i + 1],
                            )
                row0 += nr

            # Merge the engine accumulators (disjoint columns, 0 elsewhere)
            nc.vector.tensor_add(scoresT[:], scoresD[:], scoresA[:])

            # Packed output tile: col 0/1 = colmax (u=0/1), col 2 = diag,
            # col 3 = rowcost. Transposed at the end into [4, 128]. The
            # memset covers the pad lanes the transposes read.
            out_t = smpool.tile([P, 32], f32, tag="out_t")
            nc.gpsimd.memset(out_t[:], 0.0)

            # Column maxima over local rows, diagonal masked to -1e30:
            # fused (scoresT + mask) then max-reduce.
            for u in range(U):
                cscr = prpool.tile([P, BL], f32, tag="cscr")
                nc.vector.tensor_add(
                    cscr[:],
                    scoresT[:, u * BL:(u + 1) * BL],
                    mt_t[:, u * BL:(u + 1) * BL],
                )
                nc.vector.reduce_max(
                    out_t[:, u:u + 1], cscr[:], axis=mybir.AxisListType.X
                )

            # Transpose scores^T -> rows [32, 256] via 32x32 stream blocks.
            # rows[i, u*128 + pp] = score(i, j=2*pp+u).
            rows = smpool.tile([BL, B], f32, tag="rows")
            for u in range(U):
                for k in range(P // 32):
                    nc.vector.transpose(
                        out=rows[0:BL, u * P + k * 32:u * P + (k + 1) * 32],
                        in_=scoresT[k * 32:(k + 1) * 32, u * BL:(u + 1) * BL],
                    )

            # rowmax (diag masked) and diag, both fused single passes.
            rowstat = smpool.tile([BL, 4], f32, tag="rowstat")
            rscr1 = prpool.tile([BL, B], f32, tag="rscr")
            nc.vector.tensor_add(rscr1[:], rows[:], nr_t[:])
            nc.vector.reduce_max(
                rowstat[:, 0:1], rscr1[:], axis=mybir.AxisListType.X
            )
            rscr2 = prpool.tile([BL, B], f32, tag="rscr")
            # diag = sum(rows * eye) fused in one pass
            nc.vector.scalar_tensor_tensor(
                out=rscr2[:],
                in0=rows[:],
                scalar=1.0,
                in1=er_t[:],
                op0=mult,
                op1=mult,
                accum_out=out_t[0:BL, 2:3],
            )
            # rowcost = relu(margin + rowmax - diag)
            nc.vector.tensor_sub(rowstat[:, 1:2], rowstat[:, 0:1], out_t[0:BL, 2:3])
            nc.vector.tensor_scalar(
                out=out_t[0:BL, 3:4], in0=rowstat[:, 1:2],
                scalar1=MARGIN, scalar2=0.0, op0=add, op1=amax,
            )

            # Pack: transpose out_t's first 4 columns into rows of outT,
            # then ONE 4-descriptor DMA (512B per partition line).
            outT = smpool.tile([32, P], f32, tag="outT")
            for k in range(P // 32):
                nc.vector.transpose(
                    out=outT[0:32, k * 32:(k + 1) * 32],
                    in_=out_t[k * 32:(k + 1) * 32, 0:32],
                )
            nc.scalar.dma_start(out=o_d[:], in_=outT[0:4, 0:P])

    nc.compile()
    return nc


def _get_nc():
    global _NC
    if _NC is None:
        _NC = _build_nc()
    return _NC


def _make_in_maps(im, s):
    im16 = im.astype(np.float16)
    s16 = s.astype(np.float16)
    il = np.arange(BL)
    # column q in `rows` layout: q = u*128 + pp  <->  j = 2*pp + u
    jq = 2 * (np.arange(B) % P) + (np.arange(B) // P)
    in_maps = []
    for c in range(M):
        jdiag = c * BL + il                      # global row index of local i
        mt = np.zeros((P, U * BL), np.float32)   # mt[p, u*BL+i]
        pd, ud = jdiag % P, jdiag // P
        # j = 2p+u == jdiag  =>  p = jdiag//2, u = jdiag%2
        mt[jdiag // 2, (jdiag % 2) * BL + il] = NEG
        nr = np.zeros((BL, B), np.float32)
        er = np.zeros((BL, B), np.float32)
        qdiag = (jdiag % 2) * P + jdiag // 2     # q with j(q) == jdiag
        nr[il, qdiag] = NEG
        er[il, qdiag] = 1.0
        in_maps.append({
            "im": im16,
            "s": s16[c * BL:(c + 1) * BL],
            "mask_t_neg": mt,
            "neg_rows": nr,
            "eye_rows": er,
        })
    return in_maps


def _combine(results):
    colmax = np.full(B, -np.inf, np.float32)
    rowcosts = np.empty(B, np.float32)
    diag = np.empty(B, np.float32)
    for c in range(M):
        o = results[c]["out"]                    # [4, 128] fp32
        cm = np.stack([o[0], o[1]], axis=1).ravel()   # j = 2p+u
        colmax = np.maximum(colmax, cm)
        diag[c * BL:(c + 1) * BL] = o[2, :BL]
        rowcosts[c * BL:(c + 1) * BL] = o[3, :BL]
    cost_im = np.maximum(np.float32(MARGIN) + colmax - diag, np.float32(0.0))
    loss = rowcosts.sum(dtype=np.float32) + cost_im.sum(dtype=np.float32)
    return np.array(loss, dtype=np.float32)


def _run(im, s, **spmd_kwargs):
    from concourse.bass_utils import run_bass_kernel_spmd

    im = np.ascontiguousarray(np.asarray(im), dtype=np.float32)
    s = np.ascontiguousarray(np.asarray(s), dtype=np.float32)
    nc = _get_nc()
    res = run_bass_kernel_spmd(nc, _make_in_maps(im, s), list(range(M)),
                               **spmd_kwargs)
    return _combine(res.results), res


def kernel(im, s):
    loss, _ = _run(im, s)
    return loss


# revision 37
# speedup vs baseline: 1.3889x; 1.0377x over previous
"""Trainium2 Bass kernel for nn_ContrastiveLoss3DTo2D.

Reference computation (B=256, D=1024, margin=0.2):
    scores[i, j] = dot(im[j], s[i, j])                    # [B, B]
    cost_s  = sum_i relu(margin + max_{j!=i} scores[i,j] - scores[i,i])
    cost_im = sum_j relu(margin + max_{i!=j} scores[i,j] - scores[j,j])
    loss = cost_s + cost_im

Sharding: s (and the score matrix) is sharded along i across 8 cores
(32 rows each); im is replicated. Inputs are cast to fp16 on the host
(loss tolerance is 2e-2; fp16 keeps the dot-product error ~1e-4 rel)
which halves HBM traffic — the binding constraint — and doubles DVE
elementwise throughput. Each core streams its 16 MB shard and computes
the 32x256 score block with ONE fused DVE pass per (row, half):
tensor_tensor_reduce does multiply + free-axis accumulate (fp32) in a
single instruction, leaving ACT/PE idle.

Column layout: j = 2p + u (partition p in [0,128), u in {0,1}) so each
DMA descriptor is a contiguous 4 KB run (two adjacent j rows of D).
Per-core reductions produce colmax[256] (diag masked), diag[32], and
rowcost[32], packed via 32x32 stream transposes into a single [4,128]
fp32 tensor written with one 4-descriptor DMA (per-partition-column
outputs would emit hundreds of 4-byte descriptors that crawl for >10us).
The host combines per-core partials exactly as relu/max commute.
"""

import numpy as np

B = 256
D = 1024
M = 8            # cores
BL = B // M      # 32 local rows per core
P = 128          # SBUF partitions
U = 2            # j = 2p + u column interleave
MARGIN = 0.2
NEG = -1.0e30    # diagonal mask value
NEG_INIT = -3.0e38

_NC = None


def _build_nc():
    import concourse.bacc as bacc
    from concourse import mybir
    from concourse.tile import TileContext

    f32 = mybir.dt.float32
    f16 = mybir.dt.float16
    add = mybir.AluOpType.add
    mult = mybir.AluOpType.mult
    amax = mybir.AluOpType.max

    nc = bacc.Bacc(None, target_bir_lowering=False, debug=False)
    im_d = nc.declare_dram_parameter("im", [B, D], f16, isOutput=False)
    s_d = nc.declare_dram_parameter("s", [BL, B, D], f16, isOutput=False)
    mt_d = nc.declare_dram_parameter("mask_t_neg", [P, U * BL], f32, isOutput=False)
    nr_d = nc.declare_dram_parameter("neg_rows", [BL, B], f32, isOutput=False)
    er_d = nc.declare_dram_parameter("eye_rows", [BL, B], f32, isOutput=False)
    o_d = nc.declare_dram_parameter("out", [4, P], f32, isOutput=True)

    with TileContext(nc) as tc:
        with (
            tc.tile_pool(name="const", bufs=1) as cpool,
            tc.tile_pool(name="sload", bufs=5) as spool,
            tc.tile_pool(name="scratch", bufs=2) as prpool,
            tc.tile_pool(name="prods", bufs=12) as mpool,
            tc.tile_pool(name="small", bufs=1) as smpool,
            tc.psum_pool(name="pgarbF", bufs=1) as gpoolF,
            tc.psum_pool(name="pgarbA", bufs=1) as gpoolA,
        ):
            # im packed as [p, u*D + d] with j = 2p + u, matching s tiles.
            # First on the Sync ring so compute can start ~11us in; the
            # epilogue-only masks ride the scalar ring (ACT is idle early).
            im_t = cpool.tile([P, U * D], f16, tag="im")
            nc.sync.dma_start(
                out=im_t[:].rearrange("p (u d) -> p u d", u=U),
                in_=im_d[:].rearrange("(p u) d -> p u d", p=P),
            )
            mt_t = cpool.tile([P, U * BL], f32, tag="maskT")
            nc.scalar.dma_start(out=mt_t[:], in_=mt_d[:])
            nr_t = cpool.tile([BL, B], f32, tag="negrows")
            nc.scalar.dma_start(out=nr_t[:], in_=nr_d[:])
            er_t = cpool.tile([BL, B], f32, tag="eyerows")
            nc.scalar.dma_start(out=er_t[:], in_=er_d[:])

            # scores^T: partition p, free column u*BL + i  (j = 2p + u).
            # One accumulator per engine: a shared tile would serialize
            # DVE and ACT into convoys (every accum write is ordered
            # against the previous engine's write to the same tile).
            # Both start at 0; each column is written by exactly one
            # engine; the epilogue merges with a single add.
            scoresD = smpool.tile([P, U * BL], f32, tag="scoresD")
            scoresA = smpool.tile([P, U * BL], f32, tag="scoresA")
            nc.gpsimd.memset(scoresD[:], 0.0)
            nc.gpsimd.memset(scoresA[:], 0.0)
            scoresT = smpool.tile([P, U * BL], f32, tag="scoresT")

            # Ramped chunks: tiny first so the first mul starts ~1.5us
            # in and each early chunk's completion semaphore (data +
            # ~1.5us HBM receipt) fires just ahead of DVE's consumption;
            # tiny last so the stream tail is short.
            chunk_rows = [1, 2, 2, 2, 3, 4, 4, 4, 4, 2, 2, 1, 1]
            assert sum(chunk_rows) == BL

            # Per-ROW engine assignment, measured on HW:
            #   F-row: 2x DVE fused scalar_tensor_tensor (1223 + 140 ns
            #      each, 1 elem/cyc ALU-bound; garbage out -> PSUM).
            #   A-row: ONE DVE fp16 mul [P, 2*D] -> fp16 SBUF (2 elem/cyc,
            #      930 ns; covers both halves), then 2x ACT accum
            #      (1148 + 283 ns each, write-back -> PSUM bank).
            # 13 F-rows / 19 A-rows. Rows 0-2 are A so ACT's pipeline
            # starts as soon as the first mul lands (~14us) instead of
            # idling behind an opening F-row. F is clustered at the END:
            # once the stream is done DVE self-accumulates the last rows
            # while ACT drains its backlog of earlier A-rows, instead of
            # DVE idling behind ACT. (GpSimd can't help: its
            # tensor_reduce is partition-axis only, and its big muls
            # poison DVE's 2-ports mode.)
            f_rows = {3, 5, 8, 11, 14, 17, 20, 23, 26, 28, 29, 30, 31}
            assert len(f_rows) == 13 and BL - 1 in f_rows

            # All s chunks ride the Sync HWDGE ring: the scalar ring's
            # DGE is the ACT engine, and ACT is ~95% busy accumulating —
            # chunks issued there arrive late and stall DVE.
            row0 = 0
            for ci, nr in enumerate(chunk_rows):
                s_t = spool.tile([P, nr * U * D], f16, tag="s")
                ring = nc.sync
                ring.dma_start(
                    out=s_t[:, 0:nr * U * D].rearrange(
                        "p (r f) -> p r f", r=nr
                    ),
                    in_=s_d[row0:row0 + nr].rearrange(
                        "r (p u) d -> p r (u d)", p=P
                    ),
                )
                for r in range(nr):
                    i = row0 + r
                    if i in f_rows:
                        for u in range(U):
                            off = (r * U + u) * D
                            garb = gpoolF.tile([P, D], f32, tag="garbF")
                            nc.vector.scalar_tensor_tensor(
                                out=garb[:],
                                in0=s_t[:, off:off + D],
                                scalar=1.0,
                                in1=im_t[:, u * D:(u + 1) * D],
                                op0=mult,
                                op1=mult,
                                accum_out=scoresD[:, u * BL + i:u * BL + i + 1],
                            )
                    else:
                        off = r * U * D
                        prod = mpool.tile([P, U * D], f16, tag="prod")
                        nc.vector.tensor_mul(
                            prod[:],
                            s_t[:, off:off + U * D],
                            im_t[:],
                        )
                        for u in range(U):
                            garbA = gpoolA.tile([P, D], f32, tag="garbA")
                            nc.scalar.activation(
                                out=garbA[:],
                                in_=prod[:, u * D:(u + 1) * D],
                                func=mybir.ActivationFunctionType.Copy,
                                accum_out=scoresA[:, u * BL + i:u * BL + i + 1],
                            )
                row0 += nr

            # Merge the engine accumulators (disjoint columns, 0 elsewhere)
            nc.vector.tensor_add(scoresT[:], scoresD[:], scoresA[:])

            # Packed output tile: col 0/1 = colmax (u=0/1), col 2 = diag,
            # col 3 = rowcost. Transposed at the end into [4, 128]. The
            # memset covers the pad lanes the transposes read.
            out_t = smpool.tile([P, 32], f32, tag="out_t")
            nc.gpsimd.memset(out_t[:], 0.0)

            # Column maxima over local rows, diagonal masked to -1e30:
            # fused (scoresT + mask) then max-reduce.
            for u in range(U):
                cscr = prpool.tile([P, BL], f32, tag="cscr")
                nc.vector.tensor_add(
                    cscr[:],
                    scoresT[:, u * BL:(u + 1) * BL],
                    mt_t[:, u * BL:(u + 1) * BL],
                )
                nc.vector.reduce_max(
                    out_t[:, u:u + 1], cscr[:], axis=mybir.AxisListType.X
                )

            # Transpose scores^T -> rows [32, 256] via 32x32 stream blocks.
            # rows[i, u*128 + pp] = score(i, j=2*pp+u).
            rows = smpool.tile([BL, B], f32, tag="rows")
            for u in range(U):
                for k in range(P // 32):
                    nc.vector.transpose(
                        out=rows[0:BL, u * P + k * 32:u * P + (k + 1) * 32],
                        in_=scoresT[k * 32:(k + 1) * 32, u * BL:(u + 1) * BL],
                    )

            # rowmax (diag masked) and diag, both fused single passes.
            rowstat = smpool.tile([BL, 4], f32, tag="rowstat")
            rscr1 = prpool.tile([BL, B], f32, tag="rscr")
            nc.vector.tensor_add(rscr1[:], rows[:], nr_t[:])
            nc.vector.reduce_max(
                rowstat[:, 0:1], rscr1[:], axis=mybir.AxisListType.X
            )
            rscr2 = prpool.tile([BL, B], f32, tag="rscr")
            # diag = sum(rows * eye) fused in one pass
            nc.vector.scalar_tensor_tensor(
                out=rscr2[:],
                in0=rows[:],
                scalar=1.0,
                in1=er_t[:],
                op0=mult,
                op1=mult,
                accum_out=out_t[0:BL, 2:3],
            )
            # rowcost = relu(margin + rowmax - diag)
            nc.vector.tensor_sub(rowstat[:, 1:2], rowstat[:, 0:1], out_t[0:BL, 2:3])
            nc.vector.tensor_scalar(
                out=out_t[0:BL, 3:4], in0=rowstat[:, 1:2],
                scalar1=MARGIN, scalar2=0.0, op0=add, op1=amax,
            )

            # Pack: transpose out_t's first 4 columns into rows of outT,
            # then ONE 4-descriptor DMA (512B per partition line).
            outT = smpool.tile([32, P], f32, tag="outT")
            for k in range(P // 32):
                nc.vector.transpose(
                    out=outT[0:32, k * 32:(k + 1) * 32],
                    in_=out_t[k * 32:(k + 1) * 32, 0:32],
                )
            nc.scalar.dma_start(out=o_d[:], in_=outT[0:4, 0:P])

    nc.compile()
    return nc


def _get_nc():
    global _NC
    if _NC is None:
        _NC = _build_nc()
    return _NC


def _make_in_maps(im, s):
    im16 = im.astype(np.float16)
    s16 = s.astype(np.float16)
    il = np.arange(BL)
    # column q in `rows` layout: q = u*128 + pp  <->  j = 2*pp + u
    jq = 2 * (np.arange(B) % P) + (np.arange(B) // P)
    in_maps = []
    for c in range(M):
        jdiag = c * BL + il                      # global row index of local i
        mt = np.zeros((P, U * BL), np.float32)   # mt[p, u*BL+i]
        pd, ud = jdiag % P, jdiag // P
        # j = 2p+u == jdiag  =>  p = jdiag//2, u = jdiag%2
        mt[jdiag // 2, (jdiag % 2) * BL + il] = NEG
        nr = np.zeros((BL, B), np.float32)
        er = np.zeros((BL, B), np.float32)
        qdiag = (jdiag % 2) * P + jdiag // 2     # q with j(q) == jdiag
        nr[il, qdiag] = NEG
        er[il, qdiag] = 1.0
        in_maps.append({
            "im": im16,
            "s": s16[c * BL:(c + 1) * BL],
            "mask_t_neg": mt,
            "neg_rows": nr,
            "eye_rows": er,
        })
    return in_maps


def _combine(results):
    colmax = np.full(B, -np.inf, np.float32)
    rowcosts = np.empty(B, np.float32)
    diag = np.empty(B, np.float32)
    for c in range(M):
        o = results[c]["out"]                    # [4, 128] fp32
        cm = np.stack([o[0], o[1]], axis=1).ravel()   # j = 2p+u
        colmax = np.maximum(colmax, cm)
        diag[c * BL:(c + 1) * BL] = o[2, :BL]
        rowcosts[c * BL:(c + 1) * BL] = o[3, :BL]
    cost_im = np.maximum(np.float32(MARGIN) + colmax - diag, np.float32(0.0))
    loss = rowcosts.sum(dtype=np.float32) + cost_im.sum(dtype=np.float32)
    return np.array(loss, dtype=np.float32)


def _run(im, s, **spmd_kwargs):
    from concourse.bass_utils import run_bass_kernel_spmd

    im = np.ascontiguousarray(np.asarray(im), dtype=np.float32)
    s = np.ascontiguousarray(np.asarray(s), dtype=np.float32)
    nc = _get_nc()
    res = run_bass_kernel_spmd(nc, _make_in_maps(im, s), list(range(M)),
                               **spmd_kwargs)
    return _combine(res.results), res


def kernel(im, s):
    loss, _ = _run(im, s)
    return loss
